# revision 9
# baseline (speedup 1.0000x reference)
"""ConsRec-style GNN message passing on 8 Trainium2 NeuronCores, v2.

SpMM strategy: for each output-row shard, rows are degree-sorted and packed
into 32-row regions. Each region is computed by a chain of TensorE matmuls
with K=128 edge slots: lhsT is a [128, 32] block-sparse matrix holding the
(pow2-prescaled) edge values at (slot, row) positions, rhs is the [128, 64]
gathered source rows (fp8, pow2-prescaled). PSUM accumulates the chain in
fp32; 4 col-groups x 32 free-slots fill a 4-bank [128, 2048] PSUM round that
ScalarE evacuates with the inverse pow2 scale. Host does index-select packing
only; all model FLOPs (edge scaling, segment reduction, dense gg matmul,
gates/fusion) run on device.
"""
import sys
sys.path.insert(0, '/opt/trn_rl_repo')
import numpy as np
import ml_dtypes

U, I, G = 200000, 100000, 10000
D = 64
M = 8
N_HG = U + G
N_GI = G + I
HGBLK = N_HG // M
P = 128
REG = 32          # rows per region (= matmul M)
KS = 128          # contraction slots per matmul
RPR = 128         # regions per full round (4 col-groups x 32 free slots)

FP8 = ml_dtypes.float8_e4m3
BF16 = ml_dtypes.bfloat16

# pow2 prescales (exponent shifts only; undone on-device at PSUM evacuation)
SX = 64.0        # embedding tables
SV = 256.0       # edge values
SY1 = 256.0      # y1 table as L2 source
SY2 = 2048.0     # y2 table as L3 source
SGG = 8192.0     # gg_graph

LAST_EXEC_NS = None
EXEC_NS_PARTS = []
_DEBUG_META = {}


def _run(nc, in_maps, label):
    import os
    from concourse.bass_utils import run_bass_kernel_spmd
    trace = bool(os.environ.get("BASS_TRACE"))
    try:
        import antenv.axon_hooks  # noqa: F401  (NTFF profiling availability)
    except ModuleNotFoundError:
        os.environ["BASS_NEVER_TRACE"] = "1"
        trace = False
    res = run_bass_kernel_spmd(nc, in_maps, list(range(M)), trace=trace)
    if res.exec_time_ns is not None:
        EXEC_NS_PARTS.append((label, res.exec_time_ns))
    return res.results


# ---------------------------------------------------------------- planning

def plan_regions(nrows, degs_by_core, orders=None, reg=REG):
    """Pack rows into reg-row regions (degree-sorted unless orders given).
    Returns (orders[M][nreg,reg], c[nreg]) with chain lengths shared across
    cores (max over cores)."""
    nreg = -(-nrows // reg)
    npad = nreg * reg
    sums = np.zeros((M, nreg), np.int64)
    if orders is None:
        orders = []
        for k in range(M):
            deg = np.zeros(npad, np.int64)
            deg[:nrows] = degs_by_core[k]
            o = np.argsort(-deg, kind='stable')
            orders.append(o.reshape(nreg, reg))
    for k in range(M):
        deg = np.zeros(npad, np.int64)
        deg[:nrows] = degs_by_core[k]
        sums[k] = deg[orders[k]].sum(1)
    c = np.maximum(1, -(-sums.max(0) // KS))
    return orders, c


def make_rounds(nreg):
    """Split region ids into rounds of <= RPR regions."""
    rounds = []
    r = 0
    while r < nreg:
        n = min(RPR, nreg - r)
        rounds.append((r, n))
        r += n
    return rounds


def grid_maps(nreg, rounds, reg=REG):
    """Per region-slot (r, m) -> (partition, free offset) in the y layout."""
    part = np.zeros((nreg, reg), np.int64)
    foff = np.zeros((nreg, reg), np.int64)
    w = 0
    widths = []
    for (r0, n) in rounds:
        slots = -(-n // 4)
        q = np.arange(n)
        j = q & 3
        s = q >> 2
        part[r0:r0 + n] = (32 * j)[:, None] + np.arange(reg)[None, :]
        foff[r0:r0 + n] = (w + 64 * s)[:, None]
        widths.append(slots * 64)
        w += slots * 64
    return part, foff, w


def build_blobs(rows, cols, vals, src8, rpos, c, vscale, dt=FP8, reg=REG):
    """Pack one core's edges into (gT [128, nmm*64], lT [128, nmm*32]) of
    dtype dt. rows: local row ids; cols: indices into src8 (already dt);
    rpos: row -> packed pos."""
    nreg = len(c)
    mm_base = np.concatenate([[0], np.cumsum(c)[:-1]])
    nmm = int(c.sum())
    E = len(rows)
    lhsT = np.zeros((nmm, KS, reg), dt)
    slotcol = np.zeros((nmm, KS), np.int64)
    if E:
        rp = rpos[rows]
        order = np.argsort(rp, kind='stable')
        rp_s = rp[order]
        r_s = rp_s // reg
        cnt = np.bincount(r_s, minlength=nreg)
        starts = np.concatenate([[0], np.cumsum(cnt)[:-1]])
        idx = np.arange(E) - starts[r_s]
        ch = idx >> 7
        assert (ch < c[r_s]).all(), "chain overflow"
        mm = mm_base[r_s] + ch
        kk = idx & 127
        m_s = rp_s % reg
        lhsT[mm, kk, m_s] = (vals[order] * vscale).astype(dt)
        slotcol[mm, kk] = cols[order]
    g = src8[slotcol]  # [nmm, 128, 64] dt
    gT = np.ascontiguousarray(g.transpose(1, 0, 2)).reshape(P, nmm * D)
    lT = np.ascontiguousarray(lhsT.transpose(1, 0, 2)).reshape(P, nmm * reg)
    return gT, lT


def unpack_y(yc, orders, part, foff, nrows):
    """yc [128, W] -> [nrows, 64] float32 (padded rows dropped)."""
    ids = orders.reshape(-1)
    pt = part.reshape(-1)
    fo = foff.reshape(-1)
    vals = yc[pt[:, None], fo[:, None] + np.arange(D)[None, :]]
    out = np.zeros((len(ids), D), np.float32)
    out[ids] = vals.astype(np.float32)
    return out[:nrows]


def pack_table(tab_rows, orders, part, foff, width):
    """Inverse of unpack: place per-row [64] vectors into grid [128, width]."""
    out = np.zeros((P, width), np.float32)
    ids = orders.reshape(-1)
    pt = part.reshape(-1)
    fo = foff.reshape(-1)
    out[pt[:, None], fo[:, None] + np.arange(D)[None, :]] = tab_rows[ids]
    return out


# ---------------------------------------------------------------- emitters

def emit_spmm(nc, mybir, gp, psp, yp, g_d, l_d, y_d, rounds, c, scale_out,
              tag, out_f32=False, in_bf16=False, reg=REG):
    f32 = mybir.dt.float32
    odt = f32 if out_f32 else mybir.dt.bfloat16
    idt = mybir.dt.bfloat16 if in_bf16 else mybir.dt.float8e4
    mmoff = 0
    woff = 0
    out_tiles = []
    for (r0, nregs) in rounds:
        slots = -(-nregs // 4)
        cs = c[r0:r0 + nregs]
        nmm_r = int(cs.sum())
        g_t = gp.tile([P, nmm_r * D], idt, tag="spg")
        nc.sync.dma_start(g_t[:], g_d[:, mmoff * D:(mmoff + nmm_r) * D])
        l_t = gp.tile([P, nmm_r * reg], idt, tag="spl")
        nc.sync.dma_start(l_t[:], l_d[:, mmoff * reg:(mmoff + nmm_r) * reg])
        ps = psp.tile([P, slots * D], f32, tag="spps")
        i = 0
        for q in range(nregs):
            j = q & 3
            s = q >> 2
            cq = int(cs[q])
            for t in range(cq):
                nc.tensor.matmul(
                    ps[32 * j:32 * j + reg, D * s:D * s + D],
                    l_t[:, i * reg:(i + 1) * reg],
                    g_t[:, i * D:(i + 1) * D],
                    start=(t == 0), stop=(t == cq - 1),
                    tile_position=(0, 32 * j))
                i += 1
        y_t = yp.tile([P, slots * D], odt, tag=f"{tag}y")  # per-spmm tag: launch C keeps y3 resident
        nc.scalar.activation(y_t[:], ps[:],
                             mybir.ActivationFunctionType.Copy,
                             scale=scale_out)
        out_tiles.append(y_t)
        if y_d is not None:
            nc.sync.dma_start(y_d[:, woff:woff + slots * D], y_t[:])
        mmoff += nmm_r
        woff += slots * D
    return out_tiles


def spmm_program(specs):
    """One program running several packed SpMMs back to back.
    specs: list of (name, total_mm, ywidth, rounds, c, scale_out, in_bf16,
    reg)."""
    import concourse.bacc as bacc
    import concourse.tile as tile
    from concourse import mybir
    fp8 = mybir.dt.float8e4
    bf16 = mybir.dt.bfloat16
    nc = bacc.Bacc(None, target_bir_lowering=False, debug=False)
    drams = []
    for (name, total_mm, ywidth, rounds, c, scale_out, in_bf16, reg) in specs:
        idt = bf16 if in_bf16 else fp8
        g_d = nc.dram_tensor(f"gb{name}", [P, total_mm * D], idt,
                             kind="ExternalInput")
        l_d = nc.dram_tensor(f"lb{name}", [P, total_mm * reg], idt,
                             kind="ExternalInput")
        y_d = nc.dram_tensor(f"y{name}", [P, ywidth], bf16,
                             kind="ExternalOutput")
        drams.append((g_d, l_d, y_d))
    with tile.TileContext(nc) as tc:
        with (
            tc.tile_pool(name="gp", bufs=2) as gp,
            tc.tile_pool(name="yp", bufs=2) as yp,
            tc.tile_pool(name="psp", bufs=2, space="PSUM") as psp,
        ):
            for i, (name, total_mm, ywidth, rounds, c, scale_out,
                    in_bf16, reg) in enumerate(specs):
                g_d, l_d, y_d = drams[i]
                emit_spmm(nc, mybir, gp, psp, yp, g_d, l_d, y_d, rounds, c,
                          scale_out, name, in_bf16=in_bf16, reg=reg)
    nc.compile()
    return nc


# ---------------------------------------------------------------- kernel

def kernel(user_inputs, pos_groups, neg_groups,
           hg_rows, hg_cols, hg_vals,
           gi_rows, gi_cols, gi_vals,
           gg_graph,
           user_emb, item_emb, group_emb,
           hyper_w, hyper_b, lightgcn_w, lightgcn_b, overlap_w, overlap_b):
    global LAST_EXEC_NS, EXEC_NS_PARTS
    EXEC_NS_PARTS = []
    import concourse.bacc as bacc
    import concourse.tile as tile
    from concourse import mybir
    from concourse.masks import make_identity
    f32 = mybir.dt.float32
    fp8 = mybir.dt.float8e4
    bf16 = mybir.dt.bfloat16

    user_inputs = np.asarray(user_inputs).astype(np.int64)
    pos_groups = np.asarray(pos_groups).astype(np.int64)
    neg_groups = np.asarray(neg_groups).astype(np.int64)
    hg_rows = np.asarray(hg_rows).astype(np.int64)
    hg_cols = np.asarray(hg_cols).astype(np.int64)
    hg_vals = np.asarray(hg_vals).astype(np.float32)
    gi_rows = np.asarray(gi_rows).astype(np.int64)
    gi_cols = np.asarray(gi_cols).astype(np.int64)
    gi_vals = np.asarray(gi_vals).astype(np.float32)
    gg_graph = np.asarray(gg_graph).astype(np.float32)
    user_emb = np.asarray(user_emb).astype(np.float32)
    item_emb = np.asarray(item_emb).astype(np.float32)
    group_emb = np.asarray(group_emb).astype(np.float32)

    x0 = np.concatenate([user_emb, group_emb], axis=0)
    xgi = np.concatenate([group_emb, item_emb], axis=0)
    x0_8 = (x0 * SX).astype(FP8)
    xgi_8 = (xgi * SX).astype(FP8)

    # ----- output-row selections (as baseline) -----
    selU = np.unique(user_inputs)
    selG = np.unique(np.concatenate([pos_groups, neg_groups]))
    NPU = -(-max(1, -(-len(selU) // M)) // P) * P
    NPG = -(-max(1, -(-len(selG) // M)) // P) * P

    def _split_pad(arr, width):
        out = np.full((M, width), -1, np.int64)
        for k, p in enumerate(np.array_split(arr, M)):
            out[k, :len(p)] = p
        return out

    selU_sh = _split_pad(selU, NPU)
    selG_sh = _split_pad(selG, NPG)

    rm_core = np.full(N_HG, -1, np.int32)
    rm_loc = np.full(N_HG, -1, np.int32)
    for k in range(M):
        vu = selU_sh[k] >= 0
        rm_core[selU_sh[k][vu]] = k
        rm_loc[selU_sh[k][vu]] = np.nonzero(vu)[0]
        vg = selG_sh[k] >= 0
        rm_core[U + selG_sh[k][vg]] = k
        rm_loc[U + selG_sh[k][vg]] = NPU + np.nonzero(vg)[0]

    m3 = rm_core[hg_rows] >= 0
    e3rc = rm_core[hg_rows[m3]]
    e3rl = rm_loc[hg_rows[m3]].astype(np.int64)
    e3cg = hg_cols[m3]
    e3v = hg_vals[m3]

    psel = np.concatenate([selU, U + selG])
    needed2 = np.unique(np.concatenate([e3cg, psel]))
    R2 = -(-max(1, -(-len(needed2) // M)) // P) * P
    n2_sh = _split_pad(needed2, R2)
    c2_core = np.full(N_HG, -1, np.int32)
    c2_loc = np.full(N_HG, -1, np.int32)
    for k in range(M):
        v = n2_sh[k] >= 0
        c2_core[n2_sh[k][v]] = k
        c2_loc[n2_sh[k][v]] = np.nonzero(v)[0]
    c2_glob = c2_core.astype(np.int64) * R2 + c2_loc

    m2 = c2_core[hg_rows] >= 0
    e2rc = c2_core[hg_rows[m2]]
    e2rl = c2_loc[hg_rows[m2]].astype(np.int64)
    e2c = hg_cols[m2]
    e2v = hg_vals[m2]
    e1c_core = (hg_rows // HGBLK).astype(np.int32)
    e1l = hg_rows % HGBLK

    gmask = gi_rows < G
    grow = gi_rows[gmask]
    gc_core = rm_core[U + grow]
    gl = (rm_loc[U + grow] - NPU).astype(np.int64)
    mg = gc_core >= 0
    gi_r_by = [gl[(gc_core == k) & mg] for k in range(M)]
    gi_c_by = [gi_cols[gmask][(gc_core == k) & mg] for k in range(M)]
    gi_v_by = [gi_vals[gmask][(gc_core == k) & mg] for k in range(M)]

    # ================= launch A: L1 (all rows) + gi =================
    e1_by = [(e1l[e1c_core == k], hg_cols[e1c_core == k],
              hg_vals[e1c_core == k]) for k in range(M)]
    degs1 = [np.bincount(r, minlength=HGBLK) for r, _, _ in e1_by]
    ord1, c1 = plan_regions(HGBLK, degs1, reg=16)
    rounds1 = make_rounds(len(c1))
    part1, foff1, w1 = grid_maps(len(c1), rounds1, reg=16)
    rpos1 = [np.empty(len(c1) * 16, np.int64) for _ in range(M)]
    for k in range(M):
        rpos1[k][ord1[k].reshape(-1)] = np.arange(len(c1) * 16)

    degsgi = [np.bincount(r, minlength=NPG) for r in gi_r_by]
    # gi shares the L3-group packing order (computed below), so plan later.

    # ================= L3 planning (needed for gi order) ============
    e3u_by, e3g_by = [], []
    for k in range(M):
        mk = e3rc == k
        rl = e3rl[mk]
        cg = e3cg[mk]
        vv = e3v[mk]
        isu = rl < NPU
        e3u_by.append((rl[isu], cg[isu], vv[isu]))
        e3g_by.append((rl[~isu] - NPU, cg[~isu], vv[~isu]))
    degs3u = [np.bincount(r, minlength=NPU) for r, _, _ in e3u_by]
    degs3g = [np.bincount(r, minlength=NPG) for r, _, _ in e3g_by]
    ord3u, c3u = plan_regions(NPU, degs3u)
    ord3g, c3g = plan_regions(NPG, degs3g)

    # gi uses ord3g so its output grid aligns with the L3 groups grid
    _, cgi = plan_regions(NPG, degsgi, orders=ord3g)
    roundsgi = make_rounds(len(cgi))
    partg, foffg, wg = grid_maps(len(cgi), roundsgi)
    rpos3g = [np.empty(len(c3g) * REG, np.int64) for _ in range(M)]
    for k in range(M):
        rpos3g[k][ord3g[k].reshape(-1)] = np.arange(len(c3g) * REG)
    rpos3u = [np.empty(len(c3u) * REG, np.int64) for _ in range(M)]
    for k in range(M):
        rpos3u[k][ord3u[k].reshape(-1)] = np.arange(len(c3u) * REG)

    # L3-grid planning shared by the precise sel pass (same rows, same edges)
    nregu, nregg = len(c3u), len(c3g)
    c3 = np.concatenate([c3u, c3g])
    rounds3 = make_rounds(nregu + nregg)
    part3, foff3, w3 = grid_maps(nregu + nregg, rounds3)
    assert nregu % 4 == 0, "users regions must fill whole slot columns"
    US = nregu // 4          # user free slots; groups occupy the rest

    def sel_edges(k, srctab, dt, use_y2_ids):
        ru, cu, vu = e3u_by[k]
        rg, cg, vg = e3g_by[k]
        rows = np.concatenate([ru, rg + NPU])
        cols = np.concatenate([cu, cg])
        if use_y2_ids:
            cols = c2_glob[cols]
        vals = np.concatenate([vu, vg])
        rpos = np.concatenate([rpos3u[k], rpos3g[k] + nregu * REG])
        return build_blobs(rows, cols, vals, srctab, rpos, c3,
                           SV if dt is FP8 else 1.0, dt=dt)

    # ----- build launch A blobs -----
    x0_16 = x0.astype(BF16)
    xgi_16 = xgi.astype(BF16)
    gA, lA, gGI, lGI, gS, lS = [], [], [], [], [], []
    for k in range(M):
        r, cC, vv = e1_by[k]
        gT, lT = build_blobs(r, cC, vv, x0_8, rpos1[k], c1, SV, reg=16)
        gA.append(gT)
        lA.append(lT)
        gT, lT = build_blobs(gi_r_by[k], gi_c_by[k], gi_v_by[k], xgi_16,
                             rpos3g[k], cgi, 1.0, dt=BF16)
        gGI.append(gT)
        lGI.append(lT)
        gT, lT = sel_edges(k, x0_16, BF16, False)
        gS.append(gT)
        lS.append(lT)

    _DEBUG_META["A"] = dict(specs=[
        ("1", rounds1, c1, 1.0 / (SX * SV), 16),
        ("2", roundsgi, cgi, 1.0, 32),
        ("3", rounds3, c3, 1.0, 32),
    ])
    ncA = spmm_program([
        ("1", int(c1.sum()), w1, rounds1, c1, 1.0 / (SX * SV), False, 16),
        ("2", int(cgi.sum()), wg, roundsgi, cgi, 1.0, True, 32),
        ("3", int(c3.sum()), w3, rounds3, c3, 1.0, True, 32),
    ])
    mapsA = [{"gb1": gA[k], "lb1": lA[k], "gb2": gGI[k], "lb2": lGI[k],
              "gb3": gS[k], "lb3": lS[k]} for k in range(M)]
    resA = _run(ncA, mapsA, "A")

    y1_full = np.zeros((N_HG, D), np.float32)
    for k in range(M):
        yk = unpack_y(np.asarray(resA[k]["y1"]), ord1[k], part1, foff1, HGBLK)
        y1_full[k * HGBLK:(k + 1) * HGBLK] = yk
    gi_grid = [np.asarray(resA[k]["y2"]) for k in range(M)]   # [128, wg] bf16
    h1_grid = [np.asarray(resA[k]["y3"]) for k in range(M)]   # [128, w3] bf16

    # ================= launch B: L2 over needed2 rows ===============
    y1_8 = (y1_full * SY1).astype(FP8)
    e2_by = [(e2rl[e2rc == k], e2c[e2rc == k], e2v[e2rc == k])
             for k in range(M)]
    degs2 = [np.bincount(r, minlength=R2) for r, _, _ in e2_by]
    ord2, c2 = plan_regions(R2, degs2, reg=16)
    rounds2 = make_rounds(len(c2))
    part2, foff2, w2 = grid_maps(len(c2), rounds2, reg=16)
    rpos2 = [np.empty(len(c2) * 16, np.int64) for _ in range(M)]
    for k in range(M):
        rpos2[k][ord2[k].reshape(-1)] = np.arange(len(c2) * 16)

    gB, lB = [], []
    for k in range(M):
        r, cC, vv = e2_by[k]
        gT, lT = build_blobs(r, cC, vv, y1_8, rpos2[k], c2, SV, reg=16)
        gB.append(gT)
        lB.append(lT)

    _DEBUG_META["B"] = dict(specs=[("1", rounds2, c2, 1.0 / (SY1 * SV), 16)])
    ncB = spmm_program([("1", int(c2.sum()), w2, rounds2, c2,
                         1.0 / (SY1 * SV), False, 16)])
    mapsB = [{"gb1": gB[k], "lb1": lB[k]} for k in range(M)]
    resB = _run(ncB, mapsB, "B")

    y2_full = np.zeros((M * R2, D), np.float32)
    for k in range(M):
        yk = unpack_y(np.asarray(resB[k]["y1"]), ord2[k], part2, foff2, R2)
        y2_full[k * R2:(k + 1) * R2] = yk

    # ================= launch C: L3 + gg + fusion ===================
    y2_8 = (y2_full * SY2).astype(FP8)
    g3, l3 = [], []
    for k in range(M):
        gT, lT = sel_edges(k, y2_8, FP8, True)
        g3.append(gT)
        l3.append(lT)

    # grid coords for groups within the launch-C grid (offset by nregu regions)
    part3g = part3[nregu:]
    foff3g = foff3[nregu:]

    # host-packed fusion tables in grid layout
    hx, h1, h2, ggT, giF = [], [], [], [], []
    for k in range(M):
        gu = np.where(selU_sh[k] >= 0, selU_sh[k], 0)
        gg_s = np.where(selG_sh[k] >= 0, selG_sh[k], 0)
        uids = gu                       # user table rows for this core
        gids = U + gg_s
        hxu = pack_table(x0[uids], ord3u[k], part3[:nregu], foff3[:nregu], w3)
        hxg = pack_table(x0[gids], ord3g[k], part3g, foff3g, w3)
        hx_k = hxu + hxg
        h1_k = h1_grid[k].astype(np.float32)   # precise sel-rows layer-1 pass
        y2u = y2_full[c2_glob[uids]]
        y2g = y2_full[c2_glob[gids]]
        h2u = pack_table(y2u, ord3u[k], part3[:nregu], foff3[:nregu], w3)
        h2g = pack_table(y2g, ord3g[k], part3g, foff3g, w3)
        h2_k = h2u + h2g
        hx.append(hx_k)
        h1.append(h1_k)
        h2.append(h2_k)
        # ggT columns in group-grid order: column q = s*128 + p
        # group row at (part p, slot s) is ord3g[k] mapped via partg/foffg
        colrows = np.zeros(NPG, np.int64)   # grid position -> group id
        gpos = partg.reshape(-1) * (wg // D) + (foffg.reshape(-1) // D)
        # gpos in [0, 128*wg/64): linear grid index p*(slots)+s
        colrows[gpos] = gg_s[ord3g[k].reshape(-1)]
        slots_g = wg // D
        # ggT layout [G, NPG]: column index q = s*128 + p
        gsel = colrows.reshape(P, slots_g)      # [p, s]
        qorder = gsel.T.reshape(-1)             # q = s*128 + p
        ggT_k = np.ascontiguousarray(
            (gg_graph[qorder] * SGG).astype(FP8).T)   # [G, NPG]
        # pad K to whole 128-tiles and store partition-major for bulk DMA
        KT = -(-G // P)
        ggpad = np.zeros((KT * P, NPG), FP8)
        ggpad[:G] = ggT_k
        ggT.append(np.ascontiguousarray(
            ggpad.reshape(KT, P, NPG).transpose(1, 0, 2)).reshape(P, KT * NPG))
        gi_k = np.zeros((P, wg), np.float32)
        gi_k[:] = gi_grid[k].astype(np.float32)
        giF.append(gi_k)

    KT = -(-G // P)
    gemb_pad = np.zeros((KT * P, D), FP8)
    gemb_pad[:G] = (group_emb * SX).astype(FP8)
    gemb_8 = np.ascontiguousarray(
        gemb_pad.reshape(KT, P, D).transpose(1, 0, 2)).reshape(P, KT * D)
    wrep = {
        'hyper': np.tile(np.asarray(hyper_w, np.float32).reshape(1, D), (P, 1)),
        'light': np.tile(np.asarray(lightgcn_w, np.float32).reshape(1, D), (P, 1)),
        'over': np.tile(np.asarray(overlap_w, np.float32).reshape(1, D), (P, 1)),
    }
    brep = {
        'hyper': np.full((P, 1), np.asarray(hyper_b, np.float32).reshape(-1)[0], np.float32),
        'light': np.full((P, 1), np.asarray(lightgcn_b, np.float32).reshape(-1)[0], np.float32),
        'over': np.full((P, 1), np.asarray(overlap_b, np.float32).reshape(-1)[0], np.float32),
    }

    slots_g = wg // D
    nmm3 = int(c3.sum())
    ncC = bacc.Bacc(None, target_bir_lowering=False, debug=False)
    g3C = ncC.dram_tensor("gb", [P, nmm3 * D], fp8, kind="ExternalInput")
    l3C = ncC.dram_tensor("lb", [P, nmm3 * REG], fp8, kind="ExternalInput")
    hxC = ncC.dram_tensor("hx", [P, w3], f32, kind="ExternalInput")
    h1C = ncC.dram_tensor("h1", [P, w3], f32, kind="ExternalInput")
    h2C = ncC.dram_tensor("h2", [P, w3], f32, kind="ExternalInput")
    giC = ncC.dram_tensor("gio", [P, wg], f32, kind="ExternalInput")
    ggTC = ncC.dram_tensor("ggT", [P, KT * NPG], fp8, kind="ExternalInput")
    gembC = ncC.dram_tensor("gemb", [P, KT * D], fp8, kind="ExternalInput")
    wC = {n: ncC.dram_tensor(f"w_{n}", [P, D], f32, kind="ExternalInput")
          for n in wrep}
    bC = {n: ncC.dram_tensor(f"b_{n}", [P, 1], f32, kind="ExternalInput")
          for n in brep}
    usersC = ncC.dram_tensor("users_out", [P, US * D], f32,
                             kind="ExternalOutput")
    groupsC = ncC.dram_tensor("groups_out", [P, wg], f32,
                              kind="ExternalOutput")

    with tile.TileContext(ncC) as tc:
        with (
            tc.tile_pool(name="gp", bufs=2) as gp,
            tc.tile_pool(name="ggp", bufs=3) as ggp,
            tc.tile_pool(name="fus", bufs=1) as fus,
            tc.tile_pool(name="psp", bufs=1, space="PSUM") as psp,
            tc.tile_pool(name="psg", bufs=1, space="PSUM") as psg,
            tc.tile_pool(name="pst", bufs=2, space="PSUM") as pst,
        ):
            # ---- dense gg matmul (fp8, prescaled) ----
            ident = fus.tile([P, P], f32, tag="ident")
            make_identity(ncC, ident[:])
            ps_gg = psg.tile([D, NPG], f32, tag="psgg")
            gemb_t = fus.tile([P, KT * D], fp8, tag="ggl")
            ncC.sync.dma_start(gemb_t[:], gembC[:])
            CH = 16
            for c0 in range(0, KT, CH):
                cn = min(CH, KT - c0)
                rhs_t = ggp.tile([P, cn * NPG], fp8, tag="ggr")
                ncC.sync.dma_start(
                    rhs_t[:], ggTC[:, c0 * NPG:(c0 + cn) * NPG])
                for j in range(c0, c0 + cn):
                    for h0 in range(0, NPG, 512):
                        hn = min(512, NPG - h0)
                        ncC.tensor.matmul(
                            ps_gg[:, h0:h0 + hn],
                            gemb_t[:, j * D:(j + 1) * D],
                            rhs_t[:, (j - c0) * NPG + h0:
                                  (j - c0) * NPG + h0 + hn],
                            start=(j == 0), stop=(j == KT - 1))
            ggf_t = fus.tile([D, NPG], f32, tag="ggf")
            ncC.scalar.activation(ggf_t[:], ps_gg[:],
                                  mybir.ActivationFunctionType.Copy,
                                  scale=1.0 / (SGG * SX))
            b3_t = fus.tile([P, slots_g, D], f32, tag="b3")
            for t in range(slots_g):
                ps_tr = pst.tile([P, D], f32, tag="pstr")
                ncC.tensor.transpose(ps_tr[:], ggf_t[:, t * P:(t + 1) * P],
                                     ident[:D, :D])
                ncC.scalar.activation(b3_t[:, t, :], ps_tr[:],
                                      mybir.ActivationFunctionType.Copy)

            # ---- L3 spmm (y3 stays on-chip, f32) ----
            y3_tiles = emit_spmm(nc=ncC, mybir=mybir, gp=gp, psp=psp, yp=fus,
                                 g_d=g3C, l_d=l3C, y_d=None, rounds=rounds3,
                                 c=c3, scale_out=1.0 / (SY2 * SV), tag="s",
                                 out_f32=True)
            assert len(y3_tiles) == 1
            y3_t = y3_tiles[0]

            # ---- fusion ----
            NT = w3 // D
            hx_t = fus.tile([P, NT, D], f32, tag="hx")
            h1_t = fus.tile([P, NT, D], f32, tag="h1")
            h2_t = fus.tile([P, NT, D], f32, tag="h2")
            for nm, tt, dd in (("hx", hx_t, hxC), ("h1", h1_t, h1C),
                               ("h2", h2_t, h2C)):
                ncC.sync.dma_start(tt[:], dd[:].rearrange(
                    "p (t f) -> p t f", f=D))
            s1_t = fus.tile([P, NT, D], f32, tag="s1")
            ncC.vector.tensor_add(s1_t[:], hx_t[:], h1_t[:])
            s2_t = fus.tile([P, NT, D], f32, tag="s2")
            ncC.vector.tensor_add(s2_t[:], h2_t[:],
                                  y3_t[:].rearrange("p (t f) -> p t f", f=D))
            s3_t = fus.tile([P, NT, D], f32, tag="s3")
            ncC.vector.tensor_add(s3_t[:], s1_t[:], s2_t[:])
            accm_t = fus.tile([P, NT, D], f32, tag="accm")
            ncC.vector.tensor_scalar_mul(accm_t[:], s3_t[:], 0.25)
            ncC.sync.dma_start(usersC[:],
                               accm_t[:, 0:US, :].rearrange(
                                   "p t f -> p (t f)"))

            b1g_t = accm_t[:, US:, :]
            b2g_t = fus.tile([P, slots_g, D], f32, tag="b2g")
            ncC.sync.dma_start(b2g_t[:], giC[:].rearrange(
                "p (t f) -> p t f", f=D))
            w_t, bia_t = {}, {}
            for n in wrep:
                w_t[n] = fus.tile([P, D], f32, tag=f"w{n}", name=f"wt_{n}")
                ncC.sync.dma_start(w_t[n][:], wC[n][:])
                bia_t[n] = fus.tile([P, 1], f32, tag=f"bb{n}", name=f"bt_{n}")
                ncC.sync.dma_start(bia_t[n][:], bC[n][:])
            out_t = fus.tile([P, slots_g, D], f32, tag="outt")
            first = True
            for n, br in (('hyper', b1g_t), ('light', b2g_t[:]),
                          ('over', b3_t[:])):
                prod = fus.tile([P, slots_g, D], f32, tag=f"prod{n}")
                ncC.vector.tensor_tensor(
                    out=prod[:], in0=br,
                    in1=w_t[n][:, None, :].to_broadcast([P, slots_g, D]),
                    op=mybir.AluOpType.mult)
                dot = fus.tile([P, slots_g], f32, tag=f"dot{n}")
                ncC.vector.tensor_reduce(dot[:], prod[:],
                                         mybir.AxisListType.X,
                                         mybir.AluOpType.add)
                coef = fus.tile([P, slots_g], f32, tag=f"coef{n}")
                ncC.scalar.activation(coef[:], dot[:],
                                      mybir.ActivationFunctionType.Sigmoid,
                                      bias=bia_t[n][:])
                contrib = fus.tile([P, slots_g, D], f32, tag=f"ctr{n}")
                ncC.vector.tensor_tensor(
                    out=contrib[:], in0=br,
                    in1=coef[:, :, None].to_broadcast([P, slots_g, D]),
                    op=mybir.AluOpType.mult)
                if first:
                    ncC.vector.tensor_copy(out_t[:], contrib[:])
                    first = False
                else:
                    out2 = fus.tile([P, slots_g, D], f32, tag=f"out{n}")
                    ncC.vector.tensor_add(out2[:], out_t[:], contrib[:])
                    out_t = out2
            ncC.sync.dma_start(groupsC[:],
                               out_t[:].rearrange("p t f -> p (t f)"))
    ncC.compile()

    _DEBUG_META["C"] = dict(rounds=rounds3, c=c3, scale=1.0 / (SY2 * SV),
                            w=w3, US=US, wg=wg, NPG=NPG)
    mapsC = []
    for k in range(M):
        im = {"gb": g3[k], "lb": l3[k], "hx": hx[k], "h1": h1[k],
              "h2": h2[k], "gio": giF[k], "ggT": ggT[k], "gemb": gemb_8}
        for n in wrep:
            im[f"w_{n}"] = wrep[n]
            im[f"b_{n}"] = brep[n]
        mapsC.append(im)
    resC = _run(ncC, mapsC, "C")

    if EXEC_NS_PARTS:
        LAST_EXEC_NS = int(sum(t for _, t in EXEC_NS_PARTS))

    users_tab = np.zeros((U, D), np.float32)
    groups_tab = np.zeros((G, D), np.float32)
    for k in range(M):
        uo = np.asarray(resC[k]["users_out"])
        go = np.asarray(resC[k]["groups_out"])
        # users: grid over regions [0, nregu)
        uvals = unpack_y(uo, ord3u[k], part3[:nregu], foff3[:nregu], NPU)
        vu = selU_sh[k] >= 0
        users_tab[selU_sh[k][vu]] = uvals[np.nonzero(vu)[0]]
        # groups: groups_out starts at free offset 0 == grid offset US*64
        gvals = unpack_y(go, ord3g[k], part3g, foff3g - US * D, NPG)
        vg = selG_sh[k] >= 0
        groups_tab[selG_sh[k][vg]] = gvals[np.nonzero(vg)[0]]

    return (users_tab[user_inputs], groups_tab[pos_groups],
            groups_tab[neg_groups],
            user_emb[user_inputs], group_emb[pos_groups],
            group_emb[neg_groups])
